# revision 36
# baseline (speedup 1.0000x reference)
"""BatchGAT Trainium2 kernel (Bass/Tile), data-parallel over the 8 subgraphs.

Per core (1 subgraph, n=1024 nodes, 8 heads, 2 GAT layers):
  - embedding gather via indirect DMA from the replicated 100k x 64 table
  - feature-major dataflow: xT [f, n] with features on partitions
  - per head: h'T = w_h^T @ xT (PE);  t = tanh(h'T) (ACT);
    s_bcast[128, n] = (a_src broadcast) @ t  (PE outer-product trick);
    per j-chunk: d_col = t_chunk^T @ a_dst (PE);
    numerator chunks attnT[j, i]: Lrelu(s_bcast + d_col) (ACT, bias fused),
    Exp (ACT), mask-mul with adjT (DVE);
    outT = h_aug^T @ numerator (PE; ones-column yields softmax denom Z).
  - normalization deferred: Z rows gathered via tiny PE transposes into
    column form, one batched fast reciprocal, transposed back, then a
    selection-matrix matmul broadcasts 1/Z to all output rows.
  - adj is transposed once per core (u8 -> f32 convert + 64 PE transposes)
    and reused by both layers.  Layer 1 is zero-padded to fo=32 on the host
    so both layers share one code path.
  - layer outputs are restacked feature-major via constant selection-matrix
    matmuls (PE), head mean likewise; log_softmax in node-major space.
  - all constants/weights ship in one packed [128, WCOLS] tensor (single
    DMA) to keep per-instruction semaphore fan-in within ISA limits.
"""

import numpy as np

BS, N, VOCAB, EMB, FEAT = 8, 1024, 100000, 64, 64
P = 128
NCH = N // P  # 8 node chunks
H = 8
FO = 32       # per-head output features (layer 1 zero-padded to 32)
HALF = 512    # fp32 matmul free-dim limit
# head-layers whose pre-activation runs on DVE (additive mask fused)
H_HEADS = frozenset({1, 3, 5, 7, 9, 11, 13, 15})

# wpack column layout
C_IDENT = 0            # [128,128] identity
C_W0 = 128             # 8 heads x 32 cols, partitions 0..127
C_W1 = C_W0 + 8 * 32   # 8 heads x 2 kchunks x 32 cols
C_AP0 = C_W1 + 8 * 64  # 8 heads x 2 cols (a_src, a_dst), partitions 0..31
C_AP1 = C_AP0 + 16
C_MW = C_AP1 + 16      # 16 cols, partitions 0..31 (head-mean /8)
C_SEL = C_MW + 16      # 256 cols, partitions 0..7 (1/Z row select)
C_SBLK = C_SEL + 256   # 4 x 128 cols, partitions 0..31 (x1T stacking)
C_B0 = C_SBLK + 512    # 1 col, partitions 0..31
C_B1 = C_B0 + 1        # 1 col, partitions 0..15
C_NEG = C_B1 + 1       # 1 col, all partitions: -3e38
WCOLS = C_NEG + 1

_CACHE = {}


def _build(zero_b0):
    import concourse.bass as bass
    import concourse.tile as tile
    from concourse import bacc, mybir
    from contextlib import ExitStack

    dt = mybir.dt
    f32 = dt.float32
    f32r = dt.float32r
    bf16 = dt.bfloat16
    A = mybir.ActivationFunctionType
    OP = mybir.AluOpType

    nc = bacc.Bacc("TRN2", target_bir_lowering=False, debug=False)

    x_d = nc.dram_tensor("x", [P, NCH * FEAT], f32, kind="ExternalInput")
    v_d = nc.dram_tensor("verts", [P, NCH], dt.int32, kind="ExternalInput")
    adj_d = nc.dram_tensor("adj", [N, N], dt.uint8, kind="ExternalInput")
    emb_d = nc.dram_tensor("emb_w", [VOCAB, EMB], f32, kind="ExternalInput")
    wp_d = nc.dram_tensor("wpack", [P, WCOLS], f32, kind="ExternalInput")
    idb_d = nc.dram_tensor("identb", [P, P], dt.bfloat16, kind="ExternalInput")
    out_d = nc.dram_tensor("out", [N, 16], f32, kind="ExternalOutput")

    with tile.TileContext(nc) as tc, ExitStack() as ctx:
        singles = ctx.enter_context(tc.tile_pool(name="singles", bufs=1))
        stage = ctx.enter_context(tc.tile_pool(name="stage", bufs=2))
        eepool = ctx.enter_context(tc.tile_pool(name="eepool", bufs=8))
        aupool = ctx.enter_context(tc.tile_pool(name="aupool", bufs=8))
        afpool = ctx.enter_context(tc.tile_pool(name="afpool", bufs=8))
        hpool = ctx.enter_context(tc.tile_pool(name="hpool", bufs=2))
        haug = ctx.enter_context(tc.tile_pool(name="haug", bufs=3))
        big = ctx.enter_context(tc.tile_pool(name="big", bufs=2))
        oupool = ctx.enter_context(tc.tile_pool(name="oupool", bufs=8))
        epi = ctx.enter_context(tc.tile_pool(name="epi", bufs=2))
        respool = ctx.enter_context(tc.tile_pool(name="respool", bufs=8))
        pbig = ctx.enter_context(tc.tile_pool(name="pbig", bufs=2, space="PSUM"))
        psmall = ctx.enter_context(tc.tile_pool(name="psmall", bufs=2, space="PSUM"))
        pattn_pool = ctx.enter_context(tc.tile_pool(name="pattn", bufs=1, space="PSUM"))

        # ---- packed constants (single DMA) ----
        wp = singles.tile([P, WCOLS], f32, tag="wp")
        nc.sync.dma_start(out=wp[:], in_=wp_d[:, :])
        ident = wp[:, C_IDENT:C_IDENT + P]
        identb = singles.tile([P, P], bf16, tag="identb")
        nc.sync.dma_start(out=identb[:], in_=idb_d[:, :])

        # ---- stage A: x0T [128, 1024] = [x^T ; emb^T] ----
        # x arrives host-reshaped chunk-major [128, 8*64]; verts as [128, 8]
        xcols = singles.tile([P, NCH * FEAT], f32, tag="xcols")
        nc.sync.dma_start(out=xcols[:], in_=x_d[:, :])
        vts = singles.tile([P, NCH], dt.int32, tag="vts")
        nc.sync.dma_start(out=vts[:], in_=v_d[:, :])
        x0T = singles.tile([P, N], f32, tag="x0T")
        pwarm = psmall.tile([P, P], f32, tag="sm")
        nc.tensor.matmul(out=pwarm[:], lhsT=ident, rhs=ident, start=True, stop=True)
        for c in range(NCH):
            sl = slice(c * P, (c + 1) * P)
            ee = eepool.tile([P, EMB], f32, tag="ee")
            nc.gpsimd.indirect_dma_start(
                out=ee[:],
                out_offset=None,
                in_=emb_d[:, :],
                in_offset=bass.IndirectOffsetOnAxis(ap=vts[:, c:c + 1], axis=0),
            )
            xe = stage.tile([P, P], f32, tag="xe")
            nc.vector.tensor_copy(
                out=xe[:, 0:FEAT], in_=xcols[:, c * FEAT:(c + 1) * FEAT]
            )
            nc.vector.tensor_copy(out=xe[:, FEAT:P], in_=ee[:])
            px = psmall.tile([P, P], f32, tag="sm")
            nc.tensor.matmul(out=px[:], lhsT=xe[:], rhs=ident, start=True, stop=True)
            nc.vector.tensor_copy(out=x0T[:, sl], in_=px[:])

        # ---- stage B: adjT f32 [128, 8*1024]; chunk jc at cols jc*N ----
        adjT = singles.tile([P, NCH * N], bf16, tag="adjT")
        madjT = singles.tile([P, NCH * N], bf16, tag="madjT")
        af_list = []
        for ic in range(NCH):
            au = aupool.tile([P, N], dt.uint8, tag="au")
            nc.sync.dma_start(out=au[:], in_=adj_d[ic * P:(ic + 1) * P, :])
            af = afpool.tile([P, N], bf16, tag="af")
            nc.vector.tensor_copy(out=af[:], in_=au[:])
            af_list.append(af)
        # jc-outer so adjT chunk 0 (needed by the first attention chunk)
        # completes first
        for jc in range(NCH):
            for ic in range(NCH):
                pt = psmall.tile([P, P], f32, tag="sm")
                nc.tensor.matmul(
                    out=pt[:], lhsT=af_list[ic][:, jc * P:(jc + 1) * P],
                    rhs=identb[:], start=True, stop=True,
                )
                nc.scalar.activation(
                    out=adjT[:, jc * N + ic * P: jc * N + (ic + 1) * P],
                    in_=pt[:], func=A.Identity,
                )
                nc.scalar.activation(
                    out=madjT[:, jc * N + ic * P: jc * N + (ic + 1) * P],
                    in_=pt[:], func=A.Identity, scale=3.0e38, bias=wp[:, C_NEG:C_NEG + 1],
                )

        # ---- GAT layers (both padded to fo=32) ----
        fo = FO
        x1T = [
            singles.tile([P, N], f32, tag=f"x1T{k}", name=f"x1T{k}")
            for k in range(2)
        ]

        xT_in = [x0T]
        msb = None
        for li in range(2):
            kch = 1 if li == 0 else 2
            c_w = C_W0 if li == 0 else C_W1
            c_ap = C_AP0 if li == 0 else C_AP1
            ou_list = []
            for h in range(H):
                apt = wp[0:fo, c_ap + h * 2: c_ap + h * 2 + 2]
                # h'T = w_h^T @ xT   -> psum [fo, 1024]
                ph = pbig.tile([fo, N], f32, tag="big2")
                for hf in range(2):
                    fs = slice(hf * HALF, (hf + 1) * HALF)
                    for k in range(kch):
                        wcol = c_w + (h * kch + k) * fo
                        nc.tensor.matmul(
                            out=ph[:, fs],
                            lhsT=wp[:, wcol:wcol + fo],
                            rhs=xT_in[k][:, fs],
                            start=(k == 0),
                            stop=(k == kch - 1),
                        )
                # tanh for attention scores only
                tT = hpool.tile([fo, N], f32, tag="tT")
                nc.scalar.activation(out=tT[:], in_=ph[:], func=A.Tanh)
                # hT rows: 0..fo-1 h' (pre-tanh), fo = ones
                hT = hpool.tile([fo + 1, N], f32, tag="hT")
                nc.vector.tensor_copy(out=hT[0:fo, :], in_=ph[:])
                nc.gpsimd.memset(hT[fo:fo + 1, :], 1.0)
                # s broadcast to all 128 partitions: (a_src 1^T)^T @ tT
                psb = pbig.tile([P, N], f32, tag="big2")
                for hf in range(2):
                    fs = slice(hf * HALF, (hf + 1) * HALF)
                    nc.tensor.matmul(
                        out=psb[:, fs],
                        lhsT=apt[:, 0:1].to_broadcast([fo, P]),
                        rhs=tT[:, fs],
                        start=True, stop=True,
                    )

                gh = li * H + h
                h_path = gh in H_HEADS
                sbc = big.tile([P, N], f32, tag="sbc")
                nc.vector.tensor_copy(out=sbc[:], in_=psb[:])
                # attention: per j-chunk build numerator, accumulate output
                pat = pattn_pool.tile([fo + 1, N], f32, tag="pat")
                for jc in range(NCH):
                    # transpose of [h'; ones] chunk -> cols 0..fo,
                    # d column from t^T @ a_dst -> col fo+1
                    ptr = psmall.tile([P, fo + 2], f32, tag="sm")
                    nc.tensor.matmul(
                        out=ptr[:, 0:fo + 1],
                        lhsT=hT[:, jc * P:(jc + 1) * P],
                        rhs=wp[0:fo + 1, 0:fo + 1],
                        start=True, stop=True,
                    )
                    nc.tensor.matmul(
                        out=ptr[:, fo + 1:fo + 2],
                        lhsT=tT[:, jc * P:(jc + 1) * P],
                        rhs=apt[:, 1:2],
                        start=True, stop=True,
                    )
                    ha = haug.tile([P, fo + 2], bf16, tag="ha")
                    nc.vector.tensor_copy(out=ha[:], in_=ptr[:])

                    adjc = adjT[:, jc * N:(jc + 1) * N]
                    madjc = madjT[:, jc * N:(jc + 1) * N]
                    if h_path:
                        # x = s + d - BIG*(1-adj), lrelu via stt, then exp
                        xm = big.tile([P, N], f32, tag="lr", bufs=4)
                        nc.vector.scalar_tensor_tensor(
                            out=xm[:], in0=sbc[:], scalar=ha[:, fo + 1:fo + 2],
                            in1=madjc, op0=OP.add, op1=OP.add)
                        lm = big.tile([P, N], f32, tag="en", bufs=4)
                        nc.vector.scalar_tensor_tensor(
                            out=lm[:], in0=xm[:], scalar=0.2,
                            in1=xm[:], op0=OP.mult, op1=OP.max)
                        mk = big.tile([P, N], bf16, tag="mk", bufs=4)
                        nc.scalar.activation(out=mk[:], in_=lm[:], func=A.Exp)
                    else:
                        lr = big.tile([P, N], f32, tag="lr", bufs=4)
                        nc.scalar.activation(
                            out=lr[:], in_=sbc[:], func=A.Prelu,
                            bias=ha[:, fo + 1:fo + 2], scale=1.0, alpha=0.2,
                        )
                        en = big.tile([P, N], bf16, tag="en", bufs=4)
                        nc.scalar.activation(out=en[:], in_=lr[:], func=A.Exp)
                        mk = big.tile([P, N], bf16, tag="mk", bufs=4)
                        mask_eng = nc.vector if ((gh * NCH + jc) % 5 == 0) else nc.gpsimd
                        mask_eng.tensor_tensor(
                            out=mk[:], in0=en[:], in1=adjc, op=OP.mult,
                        )
                    for hf in range(2):
                        fs = slice(hf * HALF, (hf + 1) * HALF)
                        nc.tensor.matmul(
                            out=pat[:, fs],
                            lhsT=ha[:, 0:fo + 1],
                            rhs=mk[:, fs],
                            start=(jc == 0),
                            stop=(jc == NCH - 1),
                        )
                # evacuate: unscaled out rows + Z row (partition fo=32)
                ou = oupool.tile([fo + 1, N], f32, tag="ou")
                nc.vector.tensor_copy(out=ou[:], in_=pat[:])
                ou_list.append(ou)

            # gather Z rows into column form [128, 8*NCH] via PE transposes
            zcols = singles.tile([P, H * NCH], f32, tag=f"zcols{li}",
                                 name=f"zcols{li}")
            for c in range(NCH):
                pzc = psmall.tile([P, H], f32, tag="sm")
                for h in range(H):
                    nc.tensor.matmul(
                        out=pzc[:, h:h + 1],
                        lhsT=ou_list[h][fo:fo + 1, c * P:(c + 1) * P],
                        rhs=wp[fo:fo + 1, fo:fo + 1],
                        start=True, stop=True,
                    )
                nc.vector.tensor_copy(
                    out=zcols[:, c * H:(c + 1) * H], in_=pzc[:]
                )
            rcols = singles.tile([P, H * NCH], f32, tag=f"rcols{li}",
                                 name=f"rcols{li}")
            rscr = singles.tile([P, H * NCH], f32, tag=f"rscr{li}",
                                name=f"rscr{li}")
            nc.vector.reciprocal_approx_accurate(
                out=rcols[:], in_=zcols[:], scratch=rscr[:]
            )
            # transpose back to rows: rall [8, 1024]
            rall = singles.tile([H, N], f32, tag=f"rall{li}", name=f"rall{li}")
            for c in range(NCH):
                prr = psmall.tile([H, P], f32, tag="sm")
                nc.tensor.matmul(
                    out=prr[:], lhsT=rcols[:, c * H:(c + 1) * H], rhs=ident,
                    start=True, stop=True,
                )
                nc.vector.tensor_copy(
                    out=rall[:, c * P:(c + 1) * P], in_=prr[:]
                )

            xr_list = []
            pm = None
            for h in range(H):
                # broadcast 1/Z_h to fo rows via selection matrix
                prb = pbig.tile([fo, N], f32, tag="big2")
                for hf in range(2):
                    fs = slice(hf * HALF, (hf + 1) * HALF)
                    nc.tensor.matmul(
                        out=prb[:, fs],
                        lhsT=wp[0:H, C_SEL + h * fo: C_SEL + (h + 1) * fo],
                        rhs=rall[:, fs],
                        start=True, stop=True,
                    )
                y = epi.tile([fo, N], f32, tag="y")
                nc.vector.tensor_tensor(
                    out=y[:], in0=ou_list[h][0:fo, :], in1=prb[:], op=OP.mult
                )
                if li == 0:
                    # x1 rows = elu(y + b0)
                    if not zero_b0:
                        yb = epi.tile([fo, N], f32, tag="yb")
                        nc.vector.tensor_scalar(
                            out=yb[:], in0=y[:], scalar1=wp[0:fo, C_B0:C_B0 + 1],
                            scalar2=None, op0=OP.add,
                        )
                        y = yb
                    m = epi.tile([fo, N], f32, tag="m", bufs=1)
                    nc.vector.tensor_scalar(
                        out=m[:], in0=y[:], scalar1=0.0, scalar2=None, op0=OP.min
                    )
                    e = epi.tile([fo, N], f32, tag="e", bufs=1)
                    nc.scalar.activation(out=e[:], in_=m[:], func=A.Exp)
                    xr = oupool.tile([fo, N], f32, tag="ou", name="xr")
                    nc.vector.scalar_tensor_tensor(
                        out=xr[:], in0=e[:], scalar=-1.0, in1=y[:],
                        op0=OP.add, op1=OP.max,
                    )
                    xr_list.append(xr)
                else:
                    # head-mean accumulation: pm += mw^T @ y
                    if pm is None:
                        pm = pattn_pool.tile([16, N], f32, tag="pat")
                    for hf in range(2):
                        fs = slice(hf * HALF, (hf + 1) * HALF)
                        nc.tensor.matmul(
                            out=pm[:, fs],
                            lhsT=wp[0:fo, C_MW:C_MW + 16],
                            rhs=y[:, fs],
                            start=(h == 0),
                            stop=(h == H - 1),
                        )
            if li == 0:
                # restack 8 x [32, N] into 2 x [128, N] via selection matmuls
                for k in range(2):
                    px1 = pattn_pool.tile([P, N], f32, tag="pat")
                    for hf in range(2):
                        fs = slice(hf * HALF, (hf + 1) * HALF)
                        for j in range(4):
                            nc.tensor.matmul(
                                out=px1[:, fs],
                                lhsT=wp[0:fo, C_SBLK + j * P: C_SBLK + (j + 1) * P],
                                rhs=xr_list[k * 4 + j][:, fs],
                                start=(j == 0),
                                stop=(j == 3),
                            )
                    nc.vector.tensor_copy(out=x1T[k][:], in_=px1[:])
                xT_in = x1T
            else:
                msb = singles.tile([16, N], f32, tag="msb")
                nc.vector.tensor_scalar(
                    out=msb[:], in0=pm[:], scalar1=wp[0:16, C_B1:C_B1 + 1],
                    scalar2=None, op0=OP.add,
                )

        # ---- log_softmax over the 16 features, node-major ----
        for ic in range(NCH):
            pf = psmall.tile([P, 16], f32, tag="sm")
            nc.tensor.matmul(
                out=pf[:], lhsT=msb[:, ic * P:(ic + 1) * P],
                rhs=wp[0:16, 0:16],
                start=True, stop=True,
            )
            fm = stage.tile([P, 16], f32, tag="fm")
            nc.vector.tensor_copy(out=fm[:], in_=pf[:])
            nmx = stage.tile([P, 1], f32, tag="nmx")
            nc.vector.tensor_reduce(
                out=nmx[:], in_=fm[:], axis=mybir.AxisListType.X,
                op=OP.max, negate=True,
            )
            et = stage.tile([P, 16], f32, tag="et")
            se = stage.tile([P, 1], f32, tag="se")
            nc.scalar.activation(
                out=et[:], in_=fm[:], func=A.Exp, bias=nmx[:, :1],
                accum_out=se[:, :1],
            )
            lse = stage.tile([P, 1], f32, tag="lse")
            nc.scalar.activation(out=lse[:], in_=se[:], func=A.Ln)
            res = respool.tile([P, 16], f32, tag="res")
            nc.vector.tensor_scalar(
                out=res[:], in0=fm[:], scalar1=nmx[:, :1], scalar2=lse[:, :1],
                op0=OP.add, op1=OP.subtract,
            )
            nc.sync.dma_start(out=out_d[ic * P:(ic + 1) * P, :], in_=res[:])

    nc.compile()
    return nc


def _make_wpack(inputs):
    f32 = np.float32
    wpack = np.zeros((P, WCOLS), f32)
    wpack[:, C_IDENT:C_IDENT + P] = np.eye(P, dtype=f32)
    w0 = np.asarray(inputs["w0"], f32)
    for h in range(H):
        wpack[:, C_W0 + h * FO: C_W0 + (h + 1) * FO] = w0[h]
    w1 = np.asarray(inputs["w1"], f32)  # [8, 256, 16]
    for h in range(H):
        for k in range(2):
            blk = np.zeros((P, FO), f32)
            blk[:, :16] = w1[h, k * P:(k + 1) * P, :]
            wpack[:, C_W1 + (h * 2 + k) * FO: C_W1 + (h * 2 + k + 1) * FO] = blk
    a_src0 = np.asarray(inputs["a_src0"], f32)[..., 0]  # [8, 32]
    a_dst0 = np.asarray(inputs["a_dst0"], f32)[..., 0]
    a_src1 = np.asarray(inputs["a_src1"], f32)[..., 0]  # [8, 16]
    a_dst1 = np.asarray(inputs["a_dst1"], f32)[..., 0]
    for h in range(H):
        wpack[0:FO, C_AP0 + h * 2] = a_src0[h]
        wpack[0:FO, C_AP0 + h * 2 + 1] = a_dst0[h]
        wpack[0:16, C_AP1 + h * 2] = a_src1[h]
        wpack[0:16, C_AP1 + h * 2 + 1] = a_dst1[h]
    wpack[0:16, C_MW:C_MW + 16] = np.eye(16, dtype=f32) / 8.0
    wpack[0:H, C_SEL:C_SEL + H * FO] = np.kron(
        np.eye(H, dtype=f32), np.ones((1, FO), f32)
    )
    for j in range(4):
        wpack[0:FO, C_SBLK + j * P: C_SBLK + (j + 1) * P] = np.eye(
            FO, P, k=j * FO, dtype=f32
        )
    wpack[0:FO, C_B0] = np.asarray(inputs["b0"], f32).reshape(FO)
    wpack[0:16, C_B1] = np.asarray(inputs["b1"], f32).reshape(16)
    wpack[:, C_NEG] = -3.0e38
    return wpack


def _prep_inputs(inputs):
    x = np.asarray(inputs["x"], np.float32)
    verts = np.asarray(inputs["vertices"]).astype(np.int32)
    adj = np.asarray(inputs["adj"]).astype(np.uint8)
    emb_w = np.ascontiguousarray(np.asarray(inputs["emb_w"], np.float32))
    wpack = np.ascontiguousarray(_make_wpack(inputs))
    import ml_dtypes
    identb = np.ascontiguousarray(np.eye(P, dtype=ml_dtypes.bfloat16))
    in_maps = []
    for c in range(BS):
        in_maps.append({
            "x": np.ascontiguousarray(
                x[c].reshape(NCH, P, FEAT).transpose(1, 0, 2).reshape(P, NCH * FEAT)
            ),
            "verts": np.ascontiguousarray(
                verts[c].reshape(NCH, P).T
            ),
            "adj": np.ascontiguousarray(adj[c]),
            "emb_w": emb_w,
            "wpack": wpack,
            "identb": identb,
        })
    zero_b0 = bool(np.all(np.asarray(inputs["b0"]) == 0))
    return in_maps, zero_b0


def _run(inputs, trace=False):
    from concourse.bass_utils import run_bass_kernel_spmd

    in_maps, zero_b0 = _prep_inputs(inputs)
    key = ("prog", zero_b0)
    if key not in _CACHE:
        _CACHE[key] = _build(zero_b0)
    nc = _CACHE[key]
    res = run_bass_kernel_spmd(
        nc, in_maps, list(range(BS)), trace=trace
    )
    out = np.stack([res.results[c]["out"] for c in range(BS)], axis=0)
    return out.astype(np.float32), res


def kernel(**inputs):
    out, _ = _run(inputs, trace=False)
    return out
